# revision 22
# baseline (speedup 1.0000x reference)
"""Causal dot-product attention on 8 Trainium2 NeuronCores.

Problem: q,k,v [16, 2048, 128] fp32, causal softmax(q k^T / sqrt(128)) v.
Sharding: heads (N=16) split across 8 cores, 2 heads per core; no cross-core
communication.

Per-core kernel design (two heads, one per pass, pipelined):
  - Q and K are SWDGE-cast to bf16 in flight and transposed to [F, T] bf16
    layout via chunked PE transposes (matmul contraction must sit on the
    partition dim; bf16 transposes are single-pass vs fp32's dual-pass and
    the PSUM->SBUF evacuation runs in the DVE 2x packed-16-bit mode). V is
    cast to bf16 with an all-ones column appended, so the attention matmul
    itself produces the softmax row-sums.
  - Scores are computed transposed, scoresT[s, q] = kT_j.T @ qT, in pairs of
    k-tiles through 3 rotating 2-bank PSUM buffers; exp runs on the scalar
    engine (PSUM->SBUF, bf16 out, fused 1/sqrt(F) scale); the causal band of
    diagonal tiles is zeroed post-exp by a DVE multiply with a precomputed
    triangular mask (keeping gpsimd free for the SWDGE cast issues).
  - out[q, f+1] accumulates expT_ij.T @ [v_j | 1] over j into 2 PSUM banks
    (no start=True: a start clears the whole bank's has_written bits, so the
    banks are pre-zeroed and every matmul accumulates). Column 128 is the
    softmax denominator; normalize = per-partition reciprocal + scalar-mul,
    deferred off the block-boundary critical path.
  - Cold start: the first q/k chunks load as fp32 over three parallel HWDGE
    rings (sync/vector/scalar) and transpose dual-pass, so the first QK pair
    and first exp issue as early as possible; SWDGE chunk DMAs run two
    blocks ahead of their PE transposes so score production never waits on
    a transfer.
  - The final block normalizes and stores in two halves as soon as each
    half's accumulation completes, splitting the last store across two DMA
    rings to shorten the post-exp tail.
"""

import numpy as np

import concourse.bass as bass
import concourse.mybir as mybir
import concourse.tile as tile
from concourse import bacc
from concourse.bass import ts
from concourse.bass_utils import run_bass_kernel_spmd
from concourse.masks import make_identity
from concourse.tile_rust import add_dep_helper

N, T, F = 16, 2048, 128
N_CORES = 8
H = N // N_CORES  # heads per core
P = 128
NT = T // P  # 16 k/q tiles per head
BLK = 4  # q-tiles per block (512 q columns)
NBLK = NT // BLK
SCALE = 1.0 / float(np.sqrt(F))
F32 = mybir.dt.float32
F32R = mybir.dt.float32r  # TF32-like PE mode: 1 cycle/row at N>=256 (fp32 is 4)
BF16 = mybir.dt.bfloat16


def build(masked: bool):
    nc = bacc.Bacc("TRN2", target_bir_lowering=False, debug=False, num_devices=N_CORES)
    q = nc.dram_tensor("q", [H, T, F], F32, kind="ExternalInput")
    k = nc.dram_tensor("k", [H, T, F], F32, kind="ExternalInput")
    v = nc.dram_tensor("v", [H, T, F], F32, kind="ExternalInput")
    out = nc.dram_tensor("out", [H, T, F], F32, kind="ExternalOutput")

    with tile.TileContext(nc) as tc:
        _attention(tc, out, q, k, v, masked)
    nc.compile()
    return nc


def _attention(tc, out, q, k, v, masked: bool):
    from contextlib import ExitStack

    nc = tc.nc
    ctx = ExitStack()
    consts = ctx.enter_context(tc.tile_pool(name="consts", bufs=1))
    nat_pool = ctx.enter_context(tc.tile_pool(name="nat", bufs=4))
    big_pool = ctx.enter_context(tc.tile_pool(name="big", bufs=2))
    vpool = ctx.enter_context(tc.tile_pool(name="vpool", bufs=2))
    exp_pool = ctx.enter_context(tc.tile_pool(name="expp", bufs=9))
    osb_pool = ctx.enter_context(tc.tile_pool(name="osb", bufs=2))
    rec_pool = ctx.enter_context(tc.tile_pool(name="rec", bufs=4))
    ps_s = ctx.enter_context(tc.tile_pool(name="ps_s", bufs=3, space="PSUM"))
    ps_acc = ctx.enter_context(tc.tile_pool(name="ps_acc", bufs=1, space="PSUM"))

    # fp32 identity first (needed by the cold-start fp32 transposes ~1us in;
    # HWDGE cannot cast, and mixing fp32 data with a bf16 permutation operand
    # is rejected by the matmul dtype rules)
    identity_f32 = consts.tile([P, P], F32)
    nc.gpsimd.memset(identity_f32[:], 0.0)
    nc.gpsimd.affine_select(
        out=identity_f32[:],
        in_=identity_f32[:],
        compare_op=mybir.AluOpType.not_equal,
        fill=1.0,
        base=0,
        pattern=[[-1, P]],
        channel_multiplier=1,
    )
    identity = consts.tile([P, P], BF16)
    nc.gpsimd.memset(identity[:], 0.0)
    id_fill = nc.gpsimd.affine_select(  # handle kept for SWDGE dep anchoring
        out=identity[:],
        in_=identity[:],
        compare_op=mybir.AluOpType.not_equal,
        fill=1.0,
        base=0,
        pattern=[[-1, P]],
        channel_multiplier=1,
    )
    # causal keep-mask for diagonal tiles: cmask[s, q] = 1.0 if q >= s else 0
    # (applied post-exp by a DVE multiply, keeping gpsimd free for SWDGE)
    cmask = consts.tile([P, P], BF16)
    nc.gpsimd.memset(cmask[:], 1.0)
    nc.gpsimd.affine_select(
        out=cmask[:],
        in_=cmask[:],
        compare_op=mybir.AluOpType.is_ge,
        fill=0.0,
        base=0,
        pattern=[[1, P]],
        channel_multiplier=-1,
    )
    # touch Exp once at t=0 so the ~2.7us ACT table load overlaps the first
    # input DMA instead of delaying the first real exp
    warm = consts.tile([P, 1], F32)
    wsrc = consts.tile([P, P], BF16)
    nc.vector.memset(wsrc[:], 0.5)
    nc.scalar.activation(warm[:], wsrc[:, 0:1], mybir.ActivationFunctionType.Exp)

    q_ap, k_ap, v_ap, out_ap = q[:], k[:], v[:], out[:]
    CH = 4  # tiles per dma/transpose chunk (= one q-block's worth)

    def dma_chunk(st, c, anchor=None):
        """SWDGE-cast 4 natural [128,128] k and q tiles to bf16 into nat
        tiles; the PE transposes run later (emitted at a subsequent inject
        point) so score production never waits on a transfer."""
        natk = nat_pool.tile([P, CH, P], BF16, tag="nat")
        dk = nc.gpsimd.dma_start(out=natk[:], in_=st["kr3"][:, c * CH : (c + 1) * CH, :])
        if anchor is not None:
            add_dep_helper(dk.ins, anchor.ins, reason="consts before DMAs")
        natq = nat_pool.tile([P, CH, P], BF16, tag="nat")
        nc.gpsimd.dma_start(out=natq[:], in_=st["qr3"][:, c * CH : (c + 1) * CH, :])
        st["nats"][c] = (natk, natq)

    def tp_chunk(st, c):
        """PE-transpose a previously DMA'd chunk into kT/qT.  Emitted as a
        REGULAR matmul (data as stationary weights, identity streaming):
        out = nat.T @ I.  Unlike transpose-mode, this path engages the HAM
        clock gate (2.4 GHz warm vs a fixed 1.2 GHz) and counts as PE-busy,
        so the transposes cost ~56 ns instead of ~107 ns and keep the PE
        warm across chunk handoffs.  Output is fp32 PSUM; the evacuation
        copy casts to bf16."""
        natk, natq = st["nats"].pop(c)
        tpk = ps_s.tile([P, CH, P], F32, tag="s")
        for u in range(CH):
            nc.tensor.matmul(
                tpk[:, u, :], lhsT=natk[:, u, :], rhs=identity[:],
                start=True, stop=True,
            )
        nc.vector.tensor_copy(st["kT"][:, c * CH * P : (c + 1) * CH * P], tpk[:])
        tpq = ps_s.tile([P, CH, P], F32, tag="s")
        for u in range(CH):
            nc.tensor.matmul(
                tpq[:, u, :], lhsT=natq[:, u, :], rhs=identity[:],
                start=True, stop=True,
            )
        nc.vector.tensor_copy(st["qT"][:, c * CH * P : (c + 1) * CH * P], tpq[:])

    def mk_state(n):
        st = {
            "n": n,
            "kr3": k_ap[n].rearrange("(j p) f -> p j f", p=P),
            "qr3": q_ap[n].rearrange("(j p) f -> p j f", p=P),
            "vr3": v_ap[n].rearrange("(j p) f -> p j f", p=P),
            "kT": big_pool.tile([P, T], BF16, tag="kT", name="kT"),
            "qT": big_pool.tile([P, T], BF16, tag="qT", name="qT"),
            "v_aug": vpool.tile([P, NT, P + 1], BF16, tag="vaug", name="v_aug"),
            "out_sb": osb_pool.tile([P, NT, P], F32, tag="osb", name="out_sb"),
            "nats": {},
        }
        nc.vector.memset(st["v_aug"][:, :, P : P + 1], 1.0)
        return st

    def load_v_half(st, h):
        # SWDGE casts fp32 -> bf16 in flight
        nc.gpsimd.dma_start(
            out=st["v_aug"][:, h * 8 : (h + 1) * 8, 0:P],
            in_=st["vr3"][:, h * 8 : (h + 1) * 8, :],
        )

    def cold_start(st):
        """Head-0 chunk 0 via three parallel HWDGE rings, fp32 (no SWDGE
        queueing, no bf16-cast dependency): q tiles 0-3 on sync, k tiles 0-1
        on vector, k tiles 2-3 on scalar.  Dual-pass fp32 transposes; the
        evacuation copies cast to bf16.  k tiles 2-3 transpose later (at the
        first inject point) so the first QK pair and first exp issue as early
        as possible.  SWDGE chunk-1 + v-half-0 loads start right behind."""
        natq = nat_pool.tile([P, CH, P], F32, tag="natf")
        nc.sync.dma_start(out=natq[:], in_=st["qr3"][:, 0:CH, :])
        natk = nat_pool.tile([P, CH, P], F32, tag="natf")
        nc.scalar.dma_start(out=natk[:, 0:2, :], in_=st["kr3"][:, 0:2, :])
        nc.sync.dma_start(out=natk[:, 2:4, :], in_=st["kr3"][:, 2:4, :])
        tpq = ps_s.tile([P, CH, P], F32, tag="s")
        for u in range(CH):
            nc.tensor.transpose(tpq[:, u, :], natq[:, u, :], identity_f32[:])
            if u == 1:
                nc.vector.tensor_copy(st["qT"][:, 0 : 2 * P], tpq[:, 0:2, :])
        nc.vector.tensor_copy(st["qT"][:, 2 * P : 4 * P], tpq[:, 2:CH, :])
        tpk = ps_s.tile([P, 2, P], F32, tag="s")
        for u in range(2):
            nc.tensor.transpose(tpk[:, u, :], natk[:, u, :], identity_f32[:])
        nc.vector.tensor_copy(st["kT"][:, 0 : 2 * P], tpk[:])
        st["cold_natk23"] = natk
        # SWDGE prefetch: chunk 1 (k+q) and v half 0 issue behind the consts
        dma_chunk(st, 1, anchor=id_fill)
        load_v_half(st, 0)

    def cold_tp_k23(st):
        natk = st.pop("cold_natk23")
        tpk = ps_s.tile([P, 2, P], F32, tag="s")
        for u in range(2):
            nc.tensor.transpose(tpk[:, u, :], natk[:, 2 + u, :], identity_f32[:])
        nc.vector.tensor_copy(st["kT"][:, 2 * P : 4 * P], tpk[:])

    def normalize_and_store(st, acc_sb, b):
        rec4 = rec_pool.tile([P, BLK], F32, tag="rec")
        nc.vector.reciprocal(rec4[:], acc_sb[:, :, P : P + 1])
        for ii in range(BLK):
            i = BLK * b + ii
            nc.vector.tensor_scalar_mul(
                st["out_sb"][:, i, :], acc_sb[:, ii, 0:P], rec4[:, ii : ii + 1]
            )
        nc.sync.dma_start(
            out=out_ap[st["n"]].rearrange("(i p) f -> p i f", p=P)[
                :, BLK * b : BLK * (b + 1), :
            ],
            in_=st["out_sb"][:, BLK * b : BLK * (b + 1), :],
        )

    def normalize_half(st, accs, h):
        """Final-block fast path: normalize + store i-tiles [12+2h, 14+2h) as
        soon as their accumulation completes, on separate DMA rings, so the
        post-last-exp tail is just one half's worth of work."""
        sb = rec_pool.tile([P, 2, P + 1], F32, tag="nhalf")
        nc.vector.tensor_copy(sb[:], accs[:, 2 * h : 2 * h + 2, 0 : P + 1])
        rc = rec_pool.tile([P, 2], F32, tag="rech")
        nc.vector.reciprocal(rc[:], sb[:, :, P : P + 1])
        lo = (NBLK - 1) * BLK + 2 * h
        for ii in range(2):
            nc.vector.tensor_scalar_mul(
                st["out_sb"][:, lo + ii, :], sb[:, ii, 0:P], rc[:, ii : ii + 1]
            )
        eng = nc.sync if h == 0 else nc.scalar
        eng.dma_start(
            out=out_ap[st["n"]].rearrange("(i p) f -> p i f", p=P)[:, lo : lo + 2, :],
            in_=st["out_sb"][:, lo : lo + 2, :],
        )

    # ---- main loop: heads x 512-wide q blocks ----
    # j-tiles are processed in pairs through 3 rotating 2-bank PSUM score
    # buffers: QK of pair g+2, exp of pair g+1, and AV of pair g all run
    # concurrently.  SWDGE chunk DMAs run two injects ahead of their PE
    # transposes; the previous block's normalize runs mid-block, off the
    # boundary handoff.
    pending = []
    st = None
    st_next = None
    # software pipeline: each group's AV matmuls are emitted after the
    # QK+exp of the next AV_DEPTH groups, so the in-order PE queue always
    # has ready QK work (including the next block's) while exp runs
    deferred = []
    AV_DEPTH = 5

    def flush_one():
        nonlocal pending
        av_fn, last_of_block, accs_, st_, b_ = deferred.pop(0)
        av_fn()
        if last_of_block:
            # evacuate accumulators; normalize is deferred further still
            acc_sb = rec_pool.tile([P, BLK, P + 1], F32, tag="accsb", name="acc_sb")
            nc.vector.tensor_copy(acc_sb[:], accs_[:, :, 0 : P + 1])
            pending.append((st_, acc_sb, b_))

    def flush_av():
        while deferred:
            flush_one()

    def masked_inject(n, b, st):
        nonlocal st_next
        if n == 0 and b == 0:
            cold_tp_k23(st)
        if b == 0:
            tp_chunk(st, 1)
            dma_chunk(st, 2)
        elif b == 1:
            tp_chunk(st, 2)
            dma_chunk(st, 3)
            load_v_half(st, 1)
        elif b == 2:
            tp_chunk(st, 3)
            if n + 1 < H:
                st_next = mk_state(n + 1)
                dma_chunk(st_next, 0)
                load_v_half(st_next, 0)
        elif b == 3 and st_next is not None:
            tp_chunk(st_next, 0)
            dma_chunk(st_next, 1)

    for n in range(H):
        st, st_next = st_next, None
        if st is None:
            st = mk_state(n)
            if masked:
                cold_start(st)
            else:
                cold_start(st)
                cold_tp_k23(st)
                tp_chunk(st, 1)
                for c in range(2, NBLK):
                    dma_chunk(st, c)
                    tp_chunk(st, c)
                load_v_half(st, 1)
        for b in range(NBLK):
            n_j = 4 * (b + 1) if masked else NT
            # Accumulators all share 2 PSUM banks at 256-fp32 stride.
            # start=True clears the whole bank's has_written bits, so only
            # the first j=0 matmul of each BANK starts (clearing the bank);
            # the neighbour accumulator's j=0 matmul is explicitly ordered
            # after it and overwrites (its hw bit was just cleared).
            accs = ps_acc.tile([P, BLK, 256], F32, tag="acc")  # 2 PSUM banks
            bank_first = {}
            final_block = masked and n == H - 1 and b == NBLK - 1
            inject_at = max(2, (n_j // 2) & ~1)
            for g0 in range(0, n_j, 2):
                if g0 == inject_at:
                    # mid-block: previous block's normalize + chunk handoff
                    # run here, clear of the boundary handoff
                    while pending:
                        normalize_and_store(*pending.pop(0))
                    if masked:
                        masked_inject(n, b, st)
                    elif b + 1 < NBLK:
                        pass  # unmasked: all chunks already loaded
                    elif n + 1 < H:
                        st_next = mk_state(n + 1)
                        cs = st_next
                        dma_chunk(cs, 0)
                        tp_chunk(cs, 0)
                        for c in range(1, NBLK):
                            dma_chunk(cs, c)
                            tp_chunk(cs, c)
                        load_v_half(cs, 0)
                        load_v_half(cs, 1)
                gsz = min(2, n_j - g0)
                # diagonal pairs only need the causal span of columns
                col_lo = 0
                if masked and g0 - 4 * b >= 0:
                    col_lo = P * (g0 - 4 * b)
                scores = ps_s.tile([P, 2, 512], F32, tag="s")
                for r in range(gsz):
                    j = g0 + r
                    nc.tensor.matmul(
                        scores[:, r, col_lo:512],
                        lhsT=st["kT"][:, ts(j, P)],
                        rhs=st["qT"][:, 512 * b + col_lo : 512 * (b + 1)],
                        start=True,
                        stop=True,
                    )
                expT = exp_pool.tile([P, 2, 512], BF16, tag="expT")
                nc.scalar.activation(
                    expT[:, 0:gsz, col_lo:512],
                    scores[:, 0:gsz, col_lo:512],
                    mybir.ActivationFunctionType.Exp,
                    scale=SCALE,
                )
                if masked:
                    # zero the upper-triangular (non-causal) band of any
                    # diagonal tile post-exp: DVE multiply by the precomputed
                    # causal keep-mask (2x packed-bf16 mode; gpsimd stays free
                    # for SWDGE issues)
                    for r in range(gsz):
                        ii = g0 + r - 4 * b
                        if 0 <= ii < BLK:
                            nc.vector.tensor_mul(
                                expT[:, r, ts(ii, P)],
                                expT[:, r, ts(ii, P)],
                                cmask[:],
                            )
                while len(deferred) >= AV_DEPTH:
                    flush_one()

                def av_fn(expT=expT, g0=g0, gsz=gsz, accs=accs, st=st, b=b,
                          bank_first=bank_first, final_block=final_block):
                    for r in range(gsz):
                        j = g0 + r
                        for ii in range(BLK):
                            i = BLK * b + ii
                            if masked and j > i:
                                continue
                            bank = ii // 2
                            first = j == 0 and bank not in bank_first
                            m = nc.tensor.matmul(
                                accs[:, ii, 0 : P + 1],
                                lhsT=expT[:, r, ts(ii, P)],
                                rhs=st["v_aug"][:, j, :],
                                start=first,
                                stop=(j == (i if masked else NT - 1)),
                                skip_group_check=True,
                            )
                            if first:
                                bank_first[bank] = m
                            elif j == 0:
                                # the bank-clearing start above must execute
                                # before this overwrite of the cleared bank
                                add_dep_helper(
                                    m.ins,
                                    bank_first[bank].ins,
                                    reason="acc bank clear precedes neighbour j0",
                                )
                    if final_block and g0 == n_j - 4:
                        normalize_half(st, accs, 0)
                    elif final_block and g0 == n_j - 2:
                        normalize_half(st, accs, 1)

                deferred.append(
                    (av_fn, g0 + 2 >= n_j and not final_block, accs, st, b)
                )
    flush_av()
    while pending:
        normalize_and_store(*pending.pop(0))

    ctx.close()


_CACHE = {}


def _get_nc(masked: bool):
    key = bool(masked)
    if key not in _CACHE:
        _CACHE[key] = build(key)
    return _CACHE[key]


def _run(q, k, v, masked, **kwargs):
    nc = _get_nc(masked)
    q = np.ascontiguousarray(np.asarray(q, dtype=np.float32))
    k = np.ascontiguousarray(np.asarray(k, dtype=np.float32))
    v = np.ascontiguousarray(np.asarray(v, dtype=np.float32))
    in_maps = [
        {
            "q": q[c * H : (c + 1) * H],
            "k": k[c * H : (c + 1) * H],
            "v": v[c * H : (c + 1) * H],
        }
        for c in range(N_CORES)
    ]
    res = run_bass_kernel_spmd(nc, in_maps, core_ids=list(range(N_CORES)), **kwargs)
    outs = np.concatenate([r["out"] for r in res.results], axis=0)
    return outs, res


def kernel(q, k, v, masked):
    m = int(np.asarray(masked))
    outs, _ = _run(q, k, v, m != 0)
    return outs


if __name__ == "__main__":
    rng = np.random.default_rng(0)
    qq = rng.standard_normal((N, T, F), dtype=np.float32)
    kk = rng.standard_normal((N, T, F), dtype=np.float32)
    vv = rng.standard_normal((N, T, F), dtype=np.float32)
    o = kernel(qq, kk, vv, 1)
    print("out", o.shape, o.dtype, float(np.abs(o).mean()))


# revision 25
# speedup vs baseline: 1.0240x; 1.0240x over previous
"""Causal dot-product attention on 8 Trainium2 NeuronCores.

Problem: q,k,v [16, 2048, 128] fp32, causal softmax(q k^T / sqrt(128)) v.
Sharding: heads (N=16) split across 8 cores, 2 heads per core; no cross-core
communication.

Per-core kernel design (two heads, one per pass, pipelined):
  - Q and K are SWDGE-cast to bf16 in flight and transposed to [F, T] bf16
    layout via chunked PE transposes (matmul contraction must sit on the
    partition dim; bf16 transposes are single-pass vs fp32's dual-pass and
    the PSUM->SBUF evacuation runs in the DVE 2x packed-16-bit mode). V is
    cast to bf16 with an all-ones column appended, so the attention matmul
    itself produces the softmax row-sums.
  - Scores are computed transposed, scoresT[s, q] = kT_j.T @ qT, in pairs of
    k-tiles through 3 rotating 2-bank PSUM buffers; exp runs on the scalar
    engine (PSUM->SBUF, bf16 out, fused 1/sqrt(F) scale); the causal band of
    diagonal tiles is zeroed post-exp by a DVE multiply with a precomputed
    triangular mask (keeping gpsimd free for the SWDGE cast issues).
  - out[q, f+1] accumulates expT_ij.T @ [v_j | 1] over j into 2 PSUM banks
    (no start=True: a start clears the whole bank's has_written bits, so the
    banks are pre-zeroed and every matmul accumulates). Column 128 is the
    softmax denominator; normalize = per-partition reciprocal + scalar-mul,
    deferred off the block-boundary critical path.
  - Cold start: the first q/k chunks load as fp32 over three parallel HWDGE
    rings (sync/vector/scalar) and transpose dual-pass, so the first QK pair
    and first exp issue as early as possible; SWDGE chunk DMAs run two
    blocks ahead of their PE transposes so score production never waits on
    a transfer.
  - The final block normalizes and stores in two halves as soon as each
    half's accumulation completes, splitting the last store across two DMA
    rings to shorten the post-exp tail.
"""

import numpy as np

import concourse.bass as bass
import concourse.mybir as mybir
import concourse.tile as tile
from concourse import bacc
from concourse.bass import ts
from concourse.bass_utils import run_bass_kernel_spmd
from concourse.masks import make_identity
from concourse.tile_rust import add_dep_helper

N, T, F = 16, 2048, 128
N_CORES = 8
H = N // N_CORES  # heads per core
P = 128
NT = T // P  # 16 k/q tiles per head
BLK = 4  # q-tiles per block (512 q columns)
NBLK = NT // BLK
SCALE = 1.0 / float(np.sqrt(F))
F32 = mybir.dt.float32
F32R = mybir.dt.float32r  # TF32-like PE mode: 1 cycle/row at N>=256 (fp32 is 4)
BF16 = mybir.dt.bfloat16


def build(masked: bool):
    nc = bacc.Bacc("TRN2", target_bir_lowering=False, debug=False, num_devices=N_CORES)
    q = nc.dram_tensor("q", [H, T, F], F32, kind="ExternalInput")
    k = nc.dram_tensor("k", [H, T, F], F32, kind="ExternalInput")
    v = nc.dram_tensor("v", [H, T, F], F32, kind="ExternalInput")
    out = nc.dram_tensor("out", [H, T, F], F32, kind="ExternalOutput")

    with tile.TileContext(nc) as tc:
        _attention(tc, out, q, k, v, masked)
    nc.compile()
    return nc


def _attention(tc, out, q, k, v, masked: bool):
    from contextlib import ExitStack

    nc = tc.nc
    ctx = ExitStack()
    consts = ctx.enter_context(tc.tile_pool(name="consts", bufs=1))
    nat_pool = ctx.enter_context(tc.tile_pool(name="nat", bufs=4))
    big_pool = ctx.enter_context(tc.tile_pool(name="big", bufs=2))
    vpool = ctx.enter_context(tc.tile_pool(name="vpool", bufs=2))
    exp_pool = ctx.enter_context(tc.tile_pool(name="expp", bufs=8))
    osb_pool = ctx.enter_context(tc.tile_pool(name="osb", bufs=2))
    rec_pool = ctx.enter_context(tc.tile_pool(name="rec", bufs=2))
    ps_s = ctx.enter_context(tc.tile_pool(name="ps_s", bufs=3, space="PSUM"))
    ps_acc = ctx.enter_context(tc.tile_pool(name="ps_acc", bufs=1, space="PSUM"))

    # fp32 identity first (needed by the cold-start fp32 transposes ~1us in;
    # HWDGE cannot cast, and mixing fp32 data with a bf16 permutation operand
    # is rejected by the matmul dtype rules)
    identity_f32 = consts.tile([P, P], F32)
    nc.gpsimd.memset(identity_f32[:], 0.0)
    nc.gpsimd.affine_select(
        out=identity_f32[:],
        in_=identity_f32[:],
        compare_op=mybir.AluOpType.not_equal,
        fill=1.0,
        base=0,
        pattern=[[-1, P]],
        channel_multiplier=1,
    )
    identity = consts.tile([P, P], BF16)
    nc.gpsimd.memset(identity[:], 0.0)
    id_fill = nc.gpsimd.affine_select(  # handle kept for SWDGE dep anchoring
        out=identity[:],
        in_=identity[:],
        compare_op=mybir.AluOpType.not_equal,
        fill=1.0,
        base=0,
        pattern=[[-1, P]],
        channel_multiplier=1,
    )
    # causal keep-mask for diagonal tiles: cmask[s, q] = 1.0 if q >= s else 0
    # (applied post-exp by a DVE multiply, keeping gpsimd free for SWDGE)
    cmask = consts.tile([P, P], BF16)
    nc.gpsimd.memset(cmask[:], 1.0)
    nc.gpsimd.affine_select(
        out=cmask[:],
        in_=cmask[:],
        compare_op=mybir.AluOpType.is_ge,
        fill=0.0,
        base=0,
        pattern=[[1, P]],
        channel_multiplier=-1,
    )
    # touch Exp once at t=0 so the ~2.7us ACT table load overlaps the first
    # input DMA instead of delaying the first real exp
    warm = consts.tile([P, 1], F32)
    wsrc = consts.tile([P, P], BF16)
    nc.vector.memset(wsrc[:], 0.5)
    nc.scalar.activation(warm[:], wsrc[:, 0:1], mybir.ActivationFunctionType.Exp)

    q_ap, k_ap, v_ap, out_ap = q[:], k[:], v[:], out[:]
    CH = 4  # tiles per dma/transpose chunk (= one q-block's worth)

    def dma_chunk(st, c, anchor=None):
        """SWDGE-cast 4 natural [128,128] k and q tiles to bf16 into nat
        tiles; the PE transposes run later (emitted at a subsequent inject
        point) so score production never waits on a transfer."""
        natk = nat_pool.tile([P, CH, P], BF16, tag="nat")
        dk = nc.gpsimd.dma_start(out=natk[:], in_=st["kr3"][:, c * CH : (c + 1) * CH, :])
        if anchor is not None:
            add_dep_helper(dk.ins, anchor.ins, reason="consts before DMAs")
        natq = nat_pool.tile([P, CH, P], BF16, tag="nat")
        nc.gpsimd.dma_start(out=natq[:], in_=st["qr3"][:, c * CH : (c + 1) * CH, :])
        st["nats"][c] = (natk, natq)

    def tp_chunk(st, c):
        """PE-transpose a previously DMA'd chunk into kT/qT (bf16 single-pass;
        evacuation copies run in the DVE 2x packed-16-bit mode).  Measured:
        the regular-matmul transpose form (data as weights, identity
        streaming) is NOT faster in context -- the per-tile LDWEIGHTS eats
        the streaming gain -- so transpose-mode stays."""
        natk, natq = st["nats"].pop(c)
        tpk = ps_s.tile([P, CH, P], BF16, tag="s")
        for u in range(CH):
            nc.tensor.transpose(tpk[:, u, :], natk[:, u, :], identity[:])
        nc.vector.tensor_copy(st["kT"][:, c * CH * P : (c + 1) * CH * P], tpk[:])
        tpq = ps_s.tile([P, CH, P], BF16, tag="s")
        for u in range(CH):
            nc.tensor.transpose(tpq[:, u, :], natq[:, u, :], identity[:])
        nc.vector.tensor_copy(st["qT"][:, c * CH * P : (c + 1) * CH * P], tpq[:])

    def mk_state(n):
        st = {
            "n": n,
            "kr3": k_ap[n].rearrange("(j p) f -> p j f", p=P),
            "qr3": q_ap[n].rearrange("(j p) f -> p j f", p=P),
            "vr3": v_ap[n].rearrange("(j p) f -> p j f", p=P),
            "kT": big_pool.tile([P, T], BF16, tag="kT", name="kT"),
            "qT": big_pool.tile([P, T], BF16, tag="qT", name="qT"),
            "v_aug": vpool.tile([P, NT, P + 1], BF16, tag="vaug", name="v_aug"),
            "out_sb": osb_pool.tile([P, NT, P], F32, tag="osb", name="out_sb"),
            "nats": {},
        }
        nc.vector.memset(st["v_aug"][:, :, P : P + 1], 1.0)
        return st

    def load_v_half(st, h):
        # SWDGE casts fp32 -> bf16 in flight
        nc.gpsimd.dma_start(
            out=st["v_aug"][:, h * 8 : (h + 1) * 8, 0:P],
            in_=st["vr3"][:, h * 8 : (h + 1) * 8, :],
        )

    def cold_start(st):
        """Head-0 chunk 0 via three parallel HWDGE rings, fp32 (no SWDGE
        queueing, no bf16-cast dependency): q tiles 0-3 on sync, k tiles 0-1
        on vector, k tiles 2-3 on scalar.  Dual-pass fp32 transposes; the
        evacuation copies cast to bf16.  k tiles 2-3 transpose later (at the
        first inject point) so the first QK pair and first exp issue as early
        as possible.  SWDGE chunk-1 + v-half-0 loads start right behind."""
        natq = nat_pool.tile([P, CH, P], F32, tag="natf", bufs=2)
        nc.sync.dma_start(out=natq[:], in_=st["qr3"][:, 0:CH, :])
        natk = nat_pool.tile([P, CH, P], F32, tag="natf", bufs=2)
        nc.scalar.dma_start(out=natk[:, 0:2, :], in_=st["kr3"][:, 0:2, :])
        nc.sync.dma_start(out=natk[:, 2:4, :], in_=st["kr3"][:, 2:4, :])
        tpq = ps_s.tile([P, CH, P], F32, tag="s")
        for u in range(CH):
            nc.tensor.transpose(tpq[:, u, :], natq[:, u, :], identity_f32[:])
            if u == 1:
                nc.vector.tensor_copy(st["qT"][:, 0 : 2 * P], tpq[:, 0:2, :])
        nc.vector.tensor_copy(st["qT"][:, 2 * P : 4 * P], tpq[:, 2:CH, :])
        tpk = ps_s.tile([P, 2, P], F32, tag="s")
        for u in range(2):
            nc.tensor.transpose(tpk[:, u, :], natk[:, u, :], identity_f32[:])
        nc.vector.tensor_copy(st["kT"][:, 0 : 2 * P], tpk[:])
        st["cold_natk23"] = natk
        # SWDGE prefetch: chunk 1 (k+q) and v half 0 issue behind the consts
        dma_chunk(st, 1, anchor=id_fill)
        load_v_half(st, 0)

    def cold_tp_k23(st):
        natk = st.pop("cold_natk23")
        tpk = ps_s.tile([P, 2, P], F32, tag="s")
        for u in range(2):
            nc.tensor.transpose(tpk[:, u, :], natk[:, 2 + u, :], identity_f32[:])
        nc.vector.tensor_copy(st["kT"][:, 2 * P : 4 * P], tpk[:])

    def normalize_and_store(st, acc_sb, b):
        rec4 = rec_pool.tile([P, BLK], F32, tag="rec")
        nc.vector.reciprocal(rec4[:], acc_sb[:, :, P : P + 1])
        for ii in range(BLK):
            i = BLK * b + ii
            nc.vector.tensor_scalar_mul(
                st["out_sb"][:, i, :], acc_sb[:, ii, 0:P], rec4[:, ii : ii + 1]
            )
        nc.sync.dma_start(
            out=out_ap[st["n"]].rearrange("(i p) f -> p i f", p=P)[
                :, BLK * b : BLK * (b + 1), :
            ],
            in_=st["out_sb"][:, BLK * b : BLK * (b + 1), :],
        )

    def normalize_half(st, accs, h):
        """Final-block fast path: normalize + store i-tiles [12+2h, 14+2h) as
        soon as their accumulation completes, on separate DMA rings, so the
        post-last-exp tail is just one half's worth of work."""
        sb = rec_pool.tile([P, 2, P + 1], F32, tag="nhalf")
        nc.vector.tensor_copy(sb[:], accs[:, 2 * h : 2 * h + 2, 0 : P + 1])
        rc = rec_pool.tile([P, 2], F32, tag="rech")
        nc.vector.reciprocal(rc[:], sb[:, :, P : P + 1])
        lo = (NBLK - 1) * BLK + 2 * h
        for ii in range(2):
            nc.vector.tensor_scalar_mul(
                st["out_sb"][:, lo + ii, :], sb[:, ii, 0:P], rc[:, ii : ii + 1]
            )
        eng = nc.sync if h == 0 else nc.scalar
        eng.dma_start(
            out=out_ap[st["n"]].rearrange("(i p) f -> p i f", p=P)[:, lo : lo + 2, :],
            in_=st["out_sb"][:, lo : lo + 2, :],
        )

    # ---- main loop: heads x 512-wide q blocks ----
    # j-tiles are processed in pairs through 3 rotating 2-bank PSUM score
    # buffers: QK of pair g+2, exp of pair g+1, and AV of pair g all run
    # concurrently.  SWDGE chunk DMAs run two injects ahead of their PE
    # transposes; the previous block's normalize runs mid-block, off the
    # boundary handoff.
    pending = []
    st = None
    st_next = None
    # software pipeline: each group's AV matmuls are emitted after the
    # QK+exp of the next AV_DEPTH groups, so the in-order PE queue always
    # has ready QK work (including the next block's) while exp runs
    deferred = []
    AV_DEPTH = 5

    def flush_one():
        nonlocal pending
        av_fn, last_of_block, accs_, st_, b_ = deferred.pop(0)
        av_fn()
        if last_of_block:
            # evacuate accumulators; normalize is deferred further still
            acc_sb = rec_pool.tile([P, BLK, P + 1], F32, tag="accsb", name="acc_sb")
            nc.vector.tensor_copy(acc_sb[:], accs_[:, :, 0 : P + 1])
            pending.append((st_, acc_sb, b_))

    def flush_av():
        while deferred:
            flush_one()

    def masked_inject(n, b, st):
        nonlocal st_next
        if n == 0 and b == 0:
            cold_tp_k23(st)
        if b == 0:
            tp_chunk(st, 1)
            dma_chunk(st, 2)
        elif b == 1:
            tp_chunk(st, 2)
            dma_chunk(st, 3)
            load_v_half(st, 1)
        elif b == 2:
            tp_chunk(st, 3)
            if n + 1 < H:
                st_next = mk_state(n + 1)
                dma_chunk(st_next, 0)
                load_v_half(st_next, 0)
        elif b == 3 and st_next is not None:
            tp_chunk(st_next, 0)
            dma_chunk(st_next, 1)

    for n in range(H):
        st, st_next = st_next, None
        if st is None:
            st = mk_state(n)
            if masked:
                cold_start(st)
            else:
                cold_start(st)
                cold_tp_k23(st)
                tp_chunk(st, 1)
                for c in range(2, NBLK):
                    dma_chunk(st, c)
                    tp_chunk(st, c)
                load_v_half(st, 1)
        for b in range(NBLK):
            n_j = 4 * (b + 1) if masked else NT
            # Accumulators all share 2 PSUM banks at 256-fp32 stride.
            # start=True clears the whole bank's has_written bits, so only
            # the first j=0 matmul of each BANK starts (clearing the bank);
            # the neighbour accumulator's j=0 matmul is explicitly ordered
            # after it and overwrites (its hw bit was just cleared).
            accs = ps_acc.tile([P, BLK, 256], F32, tag="acc")  # 2 PSUM banks
            bank_first = {}
            final_block = masked and n == H - 1 and b == NBLK - 1
            inject_at = max(2, (n_j // 2) & ~1)
            for g0 in range(0, n_j, 2):
                if g0 == inject_at:
                    # mid-block: previous block's normalize + chunk handoff
                    # run here, clear of the boundary handoff
                    while pending:
                        normalize_and_store(*pending.pop(0))
                    if masked:
                        masked_inject(n, b, st)
                    elif b + 1 < NBLK:
                        pass  # unmasked: all chunks already loaded
                    elif n + 1 < H:
                        st_next = mk_state(n + 1)
                        cs = st_next
                        dma_chunk(cs, 0)
                        tp_chunk(cs, 0)
                        for c in range(1, NBLK):
                            dma_chunk(cs, c)
                            tp_chunk(cs, c)
                        load_v_half(cs, 0)
                        load_v_half(cs, 1)
                gsz = min(2, n_j - g0)
                # diagonal pairs only need the causal span of columns
                col_lo = 0
                if masked and g0 - 4 * b >= 0:
                    col_lo = P * (g0 - 4 * b)
                scores = ps_s.tile([P, 2, 512], F32, tag="s")
                for r in range(gsz):
                    j = g0 + r
                    nc.tensor.matmul(
                        scores[:, r, col_lo:512],
                        lhsT=st["kT"][:, ts(j, P)],
                        rhs=st["qT"][:, 512 * b + col_lo : 512 * (b + 1)],
                        start=True,
                        stop=True,
                    )
                expT = exp_pool.tile([P, 2, 512], BF16, tag="expT")
                nc.scalar.activation(
                    expT[:, 0:gsz, col_lo:512],
                    scores[:, 0:gsz, col_lo:512],
                    mybir.ActivationFunctionType.Exp,
                    scale=SCALE,
                )
                if masked:
                    # zero the upper-triangular (non-causal) band of any
                    # diagonal tile post-exp: DVE multiply by the precomputed
                    # causal keep-mask (2x packed-bf16 mode; gpsimd stays free
                    # for SWDGE issues)
                    for r in range(gsz):
                        ii = g0 + r - 4 * b
                        if 0 <= ii < BLK:
                            nc.vector.tensor_mul(
                                expT[:, r, ts(ii, P)],
                                expT[:, r, ts(ii, P)],
                                cmask[:],
                            )
                while len(deferred) >= AV_DEPTH:
                    flush_one()

                def av_fn(expT=expT, g0=g0, gsz=gsz, accs=accs, st=st, b=b,
                          bank_first=bank_first, final_block=final_block):
                    for r in range(gsz):
                        j = g0 + r
                        for ii in range(BLK):
                            i = BLK * b + ii
                            if masked and j > i:
                                continue
                            bank = ii // 2
                            first = j == 0 and bank not in bank_first
                            m = nc.tensor.matmul(
                                accs[:, ii, 0 : P + 1],
                                lhsT=expT[:, r, ts(ii, P)],
                                rhs=st["v_aug"][:, j, :],
                                start=first,
                                stop=(j == (i if masked else NT - 1)),
                                skip_group_check=True,
                            )
                            if first:
                                bank_first[bank] = m
                            elif j == 0:
                                # the bank-clearing start above must execute
                                # before this overwrite of the cleared bank
                                add_dep_helper(
                                    m.ins,
                                    bank_first[bank].ins,
                                    reason="acc bank clear precedes neighbour j0",
                                )
                    if final_block and g0 == n_j - 4:
                        normalize_half(st, accs, 0)
                    elif final_block and g0 == n_j - 2:
                        normalize_half(st, accs, 1)

                deferred.append(
                    (av_fn, g0 + 2 >= n_j and not final_block, accs, st, b)
                )
    flush_av()
    while pending:
        normalize_and_store(*pending.pop(0))

    ctx.close()


_CACHE = {}


def _get_nc(masked: bool):
    key = bool(masked)
    if key not in _CACHE:
        _CACHE[key] = build(key)
    return _CACHE[key]


def _run(q, k, v, masked, **kwargs):
    nc = _get_nc(masked)
    q = np.ascontiguousarray(np.asarray(q, dtype=np.float32))
    k = np.ascontiguousarray(np.asarray(k, dtype=np.float32))
    v = np.ascontiguousarray(np.asarray(v, dtype=np.float32))
    in_maps = [
        {
            "q": q[c * H : (c + 1) * H],
            "k": k[c * H : (c + 1) * H],
            "v": v[c * H : (c + 1) * H],
        }
        for c in range(N_CORES)
    ]
    res = run_bass_kernel_spmd(nc, in_maps, core_ids=list(range(N_CORES)), **kwargs)
    outs = np.concatenate([r["out"] for r in res.results], axis=0)
    return outs, res


def kernel(q, k, v, masked):
    m = int(np.asarray(masked))
    outs, _ = _run(q, k, v, m != 0)
    return outs


if __name__ == "__main__":
    rng = np.random.default_rng(0)
    qq = rng.standard_normal((N, T, F), dtype=np.float32)
    kk = rng.standard_normal((N, T, F), dtype=np.float32)
    vv = rng.standard_normal((N, T, F), dtype=np.float32)
    o = kernel(qq, kk, vv, 1)
    print("out", o.shape, o.dtype, float(np.abs(o).mean()))
